# revision 8
# baseline (speedup 1.0000x reference)
"""Distributed real-vector SHT on 8 Trainium2 NeuronCores.

Full inputs in, full output out. Internally: parity-split azimuthal-mode
model parallelism. Cores 0-3 take the 181 even m (46+45+45+45), cores 4-7
the 180 odd m (45 each). A mode of parity p only needs the folded input
e/o(n) = x(n) +/- x(n+360) (n < 360), so each core DMAs HALF of x and the
DFT contraction is 360-long instead of 720.

  stage 1 (DFT):  psum[lat, m, trig] += eo[lon, lat]^T @ trig[lon, m]
                  (x-tile is the STATIONARY operand, so the output lands
                  lat-major — no PE transposes needed)
  scatter:        psum -> ytA = [C0, -S0, -C1, S1], ytB = [S1, C1, S0, C0]
                  (plane arrangements with signs folded in)
  stage 2 (Leg):  out[(grp,ch), l] = sum_lat ytA*w0 + ytB*w1   (PSUM
                  accumulation performs the complex recombination for free)

out rows (grp,ch): grp0=Re(out0), grp1=Im(out0), grp2=Re(out1), grp3=Im(out1).
All tensor math happens on-device; the host does the parity fold, layout
shuffles, dtype casts and the final complex packing.
"""

import sys
import numpy as np
from contextlib import ExitStack

sys.path.insert(0, "/opt/trn_rl_repo")

import concourse.bass as bass  # noqa: E402
import concourse.tile as tile  # noqa: E402
from concourse import bacc  # noqa: E402
from concourse import mybir  # noqa: E402
from concourse.bass_utils import run_bass_kernel_spmd  # noqa: E402

NLAT, NLON = 360, 720
LMAX, MMAX = 360, 361
NCORES = 8
MPC = 46           # modes per core (padded; even: 46/45/45/45, odd: 45 x4)
CH = 32
KC = 120           # partition chunk
F16 = mybir.dt.float16
F32 = mybir.dt.float32

_CACHE = {}


def _build_program(reps=1, mode="full"):
    nc = bacc.Bacc("TRN2", target_bir_lowering=False, debug=False,
                   num_devices=NCORES)
    xt = nc.dram_tensor("xt", [KC, 3, 64, 360], F16, kind="ExternalInput").ap()
    dftm = nc.dram_tensor("dftm", [KC, 3, MPC, 2], F16,
                          kind="ExternalInput").ap()
    # weights in blocks of 4 modes so stage 2 gets large contiguous DMAs
    # and long uninterrupted PE runs (lets the PE ramp to max p-state)
    wts = nc.dram_tensor("wts", [12, KC, 4, 3, 2 * LMAX], F16,
                         kind="ExternalInput").ap()
    out = nc.dram_tensor("out", [128, 48, LMAX], F16,
                         kind="ExternalOutput").ap()

    with tile.TileContext(nc) as tc, ExitStack() as ctx:
        const_pool = ctx.enter_context(tc.tile_pool(name="const", bufs=1))
        yt_pool = ctx.enter_context(tc.tile_pool(name="yt", bufs=1))

        df_t = const_pool.tile([KC, 3, MPC, 2], F16, tag="df")
        nc.gpsimd.dma_start(df_t[:], dftm)

        # ytA/ytB: [lat-in-chunk, lat-chunk, j(padded to 48), grp4, ch32]
        ytA = yt_pool.tile([KC, 3, 48, 4, CH], F16, tag="ytA", name="ytA")
        ytB = yt_pool.tile([KC, 3, 48, 4, CH], F16, tag="ytB", name="ytB")

        if mode in ("dma", "nodve"):
            nc.gpsimd.memset(ytA[:], 0.0)
            nc.gpsimd.memset(ytB[:], 0.0)
        else:
            # pad modes 46-47 read by stage 2 but never written by stage 1
            nc.gpsimd.memset(ytA[:, :, MPC:48], 0.0)
            nc.gpsimd.memset(ytB[:, :, MPC:48], 0.0)
        for _rep in range(reps):
            _build_body(nc, tc, xt, wts, out, df_t, ytA, ytB, mode)

    nc.compile()
    return nc


def _build_body(nc, tc, xt, wts, out, df_t, ytA, ytB, mode="full"):
    dma_only = (mode == "dma")
    no_dve = (mode in ("dma", "nodve"))

    # ---- stage 1: DFT (+ scatter into ytA/ytB) ----
    with tc.tile_pool(name="xin", bufs=3) as xin_pool, \
         tc.tile_pool(name="dps", bufs=2, space="PSUM") as dps_pool:
        for g in range(16):             # 4-channel pair groups
            comp = g // 8
            ch0 = (g % 8) * 4
            x_t = xin_pool.tile([KC, 3, 4, 360], F16, tag="xin")
            nc.gpsimd.dma_start(x_t[:], xt[:, :, 4 * g:4 * g + 4, :])

            # [lat, lat-chunk(lb), ci, m(pad64), trig]
            ps = dps_pool.tile([KC, 3, 4, 64, 2], F32, tag="dps")
            for lb in range(3 if not dma_only else 0):
                for ci in range(4):
                    for kc in range(3):
                        nc.tensor.matmul(
                            ps[:, lb, ci, 0:MPC, :],
                            lhsT=x_t[:, kc, ci, lb * KC:(lb + 1) * KC],
                            rhs=df_t[:, kc],
                            start=(kc == 0), stop=(kc == 2),
                        )
            if no_dve:
                continue
            # psum -> yt scatter (signs folded); src [120, lb, j, ci]
            srcC = ps[:, :, :, 0:MPC, 0].transpose([0, 1, 3, 2])
            srcS = ps[:, :, :, 0:MPC, 1].transpose([0, 1, 3, 2])
            if comp == 0:
                # A: [C0, -S0, ...]   B: [..., S0, C0]
                nc.vector.tensor_scalar_mul(
                    ytA[:, :, 0:MPC, 1, ch0:ch0 + 4], srcS, -1.0)
                nc.vector.tensor_copy(ytB[:, :, 0:MPC, 2, ch0:ch0 + 4], srcS)
                nc.scalar.copy(ytA[:, :, 0:MPC, 0, ch0:ch0 + 4], srcC)
                nc.scalar.copy(ytB[:, :, 0:MPC, 3, ch0:ch0 + 4], srcC)
            else:
                # A: [..., -C1, S1]   B: [S1, C1, ...]
                nc.vector.tensor_scalar_mul(
                    ytA[:, :, 0:MPC, 2, ch0:ch0 + 4], srcC, -1.0)
                nc.vector.tensor_copy(ytB[:, :, 0:MPC, 1, ch0:ch0 + 4], srcC)
                nc.scalar.copy(ytA[:, :, 0:MPC, 3, ch0:ch0 + 4], srcS)
                nc.scalar.copy(ytB[:, :, 0:MPC, 0, ch0:ch0 + 4], srcS)

    # ---- stage 2: Legendre + combine-in-PSUM ----
    with tc.tile_pool(name="win", bufs=4) as w_pool, \
         tc.tile_pool(name="lps", bufs=4, space="PSUM") as lps_pool, \
         tc.tile_pool(name="osb", bufs=3) as o_pool:
        for b in range(12):
            w_t = w_pool.tile([KC, 4, 3, 2 * LMAX], F16, tag="win")
            nc.gpsimd.dma_start(w_t[:], wts[b])
            osb = o_pool.tile([128, 4, LMAX], F16, tag="osb")
            for jj in range(4):
                j = 4 * b + jj
                lp = lps_pool.tile([128, 512], F32, tag="lps")
                for lb in range(3 if not dma_only else 0):
                    nc.tensor.matmul(
                        lp[:, 0:LMAX], lhsT=ytA[:, lb, j],
                        rhs=w_t[:, jj, lb, 0:LMAX],
                        start=(lb == 0), stop=False,
                    )
                for lb in range(3 if not dma_only else 0):
                    nc.tensor.matmul(
                        lp[:, 0:LMAX], lhsT=ytB[:, lb, j],
                        rhs=w_t[:, jj, lb, LMAX:2 * LMAX],
                        start=False, stop=(lb == 2),
                    )
                if no_dve:
                    if jj == 0 and dma_only:
                        nc.gpsimd.memset(osb[:], 0.0)
                elif jj % 2 == 0:
                    nc.vector.tensor_copy(osb[:, jj, :], lp[:, 0:LMAX])
                else:
                    nc.scalar.copy(osb[:, jj, :], lp[:, 0:LMAX])
            nc.gpsimd.dma_start(out[:, 4 * b:4 * b + 4, :], osb[:])


def _mode_sets():
    even_m = np.arange(0, MMAX, 2)
    odd_m = np.arange(1, MMAX, 2)
    sets, o = [], 0
    for cnt in (46, 45, 45, 45):
        sets.append(even_m[o:o + cnt]); o += cnt
    o = 0
    for cnt in (45, 45, 45, 45):
        sets.append(odd_m[o:o + cnt]); o += cnt
    return sets


def _prep_in_maps(x, weights):
    x = np.asarray(x, dtype=np.float32)
    weights = np.asarray(weights, dtype=np.float32)

    xf = x[0]                                  # (ch, comp, lat, lon)
    e = xf[..., :360] + xf[..., 360:]
    o = xf[..., :360] - xf[..., 360:]

    def pack_x(src):
        # -> xt[p, kc, comp*32+ch, lat], lon = kc*120 + p
        a = src.transpose(3, 1, 0, 2)          # (lon, comp, ch, lat)
        a = a.reshape(3, KC, 2, CH, 360).transpose(1, 0, 2, 3, 4)
        return np.ascontiguousarray(
            a.reshape(KC, 3, 64, 360)).astype(np.float16)

    xt_eo = [pack_x(e), pack_x(o)]

    s = 2.0 * np.pi / NLON
    n = np.arange(360, dtype=np.float64)
    in_maps = []
    for c, ms in enumerate(_mode_sets()):
        nm = len(ms)
        ang = 2.0 * np.pi * np.outer(n, ms.astype(np.float64)) / NLON
        dft = np.zeros((360, MPC, 2), dtype=np.float64)
        dft[:, :nm, 0] = np.cos(ang) * s
        dft[:, :nm, 1] = np.sin(ang) * s
        dftm = np.ascontiguousarray(
            dft.reshape(3, KC, MPC, 2).transpose(1, 0, 2, 3)
        ).astype(np.float16)

        wc = np.zeros((2, 48, LMAX, NLAT), dtype=np.float32)
        wc[:, :nm] = weights[:, ms]
        tmp = wc.transpose(1, 3, 0, 2)                  # (j, lat, i, l)
        tmp = tmp.reshape(48, 3, KC, 2, LMAX).transpose(0, 2, 1, 3, 4)
        w48 = tmp.reshape(48, KC, 3, 2 * LMAX)
        w_c = np.ascontiguousarray(
            w48.reshape(12, 4, KC, 3, 2 * LMAX).transpose(0, 2, 1, 3, 4)
        ).astype(np.float16)

        in_maps.append({"xt": xt_eo[c // 4], "dftm": dftm, "wts": w_c})
    return in_maps


def _assemble(results):
    full = np.empty((1, CH, 2, LMAX, MMAX), dtype=np.complex64)
    for c, ms in enumerate(_mode_sets()):
        nm = len(ms)
        o = results[c]["out"].astype(np.float32)   # (128, 48, 360)
        g = o.reshape(4, CH, 48, LMAX)
        out0 = (g[0] + 1j * g[1]).astype(np.complex64)   # (ch, j, l)
        out1 = (g[2] + 1j * g[3]).astype(np.complex64)
        full[0, :, 0, :, ms] = out0[:, :nm].transpose(1, 0, 2)
        full[0, :, 1, :, ms] = out1[:, :nm].transpose(1, 0, 2)
    return full


def _run(x, weights, trace=False):
    if "nc" not in _CACHE:
        _CACHE["nc"] = _build_program()
    nc = _CACHE["nc"]
    in_maps = _prep_in_maps(x, weights)
    res = run_bass_kernel_spmd(nc, in_maps, list(range(NCORES)), trace=trace)
    return _assemble(res.results), res


def kernel(x, weights):
    out, _ = _run(x, weights, trace=False)
    return out
